# revision 3
# baseline (speedup 1.0000x reference)
"""CRPS loss kernel for Trainium2 (8 NeuronCores, SPMD).

Math: with |a-b| = 2*max(a,b) - a - b, for forecasts x_i (i<N) and obs y:
  T1 = sum_s sum_i |x_i - y|    = 2*Q - U - N*V
  T2 = sum_s sum_ij |x_i - x_j| = 4*Pm + (2-2N)*U
where
  Pm = sum_s sum_{i<j} max(x_i, x_j)    (device, pairwise-max over ensemble)
  Q  = sum_s sum_i max(x_i, y)          (device)
  U  = sum_s sum_i x_i,  V = sum_s y    (host, exact fp64 over fp16 inputs)
and crps_mean = T1/(N*S) - T2/(2*N^2*S).

max() is exact in fp16, so the only precision loss is the fp16 input
rounding (measured rel err ~4e-7 vs the fp32 reference).

Device layout per core: spatial shard of S_LOC=65536 points as [128, 512]
tiles; ensemble member i occupies free columns [i*512, (i+1)*512) of one
big SBUF tile. Pairwise maxes are batched per ensemble-offset diagonal
(19 DVE tensor_max ops); reduction is done on the otherwise-idle PE via
ones-vector matmuls accumulating into PSUM.
"""

import numpy as np

N_CORES = 8
N = 20
S_FULL = 4 * 1 * 8 * 128 * 128  # 524288
S_LOC = S_FULL // N_CORES  # 65536
P = 128
F = S_LOC // P  # 512

_CACHE = {}


def _build():
    import concourse.bacc as bacc
    import concourse.tile as tile
    import concourse.mybir as mybir

    f16 = mybir.dt.float16
    f32 = mybir.dt.float32

    nc = bacc.Bacc("TRN2", target_bir_lowering=False, debug=False, num_devices=N_CORES)
    x_d = nc.dram_tensor("x", [N, S_LOC], f16, kind="ExternalInput")
    y_d = nc.dram_tensor("y", [S_LOC], f16, kind="ExternalInput")
    out_d = nc.dram_tensor("out", [2, F], f32, kind="ExternalOutput")

    with tile.TileContext(nc) as tc:
        with (
            tc.tile_pool(name="data", bufs=1) as data,
            tc.tile_pool(name="scr", bufs=3) as scrp,
            tc.tile_pool(name="psum", bufs=1, space="PSUM") as pp,
        ):
            Xt = data.tile([P, N * F], f16)
            Yt = data.tile([P, F], f16)
            ones = data.tile([P, 1], f16)
            nc.vector.memset(ones[:], 1.0)

            x3 = x_d.ap().rearrange("n (p f) -> p n f", p=P)
            Xt3 = Xt[:].rearrange("p (n f) -> p n f", f=F)
            G = 4
            for g in range(G):
                lo, hi = g * (N // G), (g + 1) * (N // G)
                nc.sync.dma_start(out=Xt3[:, lo:hi, :], in_=x3[:, lo:hi, :])
            nc.sync.dma_start(
                out=Yt[:], in_=y_d.ap().rearrange("(p f) -> p f", p=P)
            )

            psum_pairs = pp.tile([1, F], f32)
            psum_obs = pp.tile([1, F], f32)

            n_pair_mm = sum(N - d for d in range(1, N))  # 190
            k = 0
            for d in range(1, N):
                L = (N - d) * F
                s = scrp.tile([P, N * F], f16, tag="scr")
                nc.vector.tensor_max(s[:, :L], Xt[:, :L], Xt[:, d * F : d * F + L])
                for j in range(N - d):
                    nc.tensor.matmul(
                        psum_pairs[:],
                        ones[:],
                        s[:, j * F : (j + 1) * F],
                        start=(k == 0),
                        stop=(k == n_pair_mm - 1),
                    )
                    k += 1

            so = scrp.tile([P, N * F], f16, tag="scr")
            so3 = so[:].rearrange("p (n f) -> p n f", f=F)
            yb = Yt[:].unsqueeze(1).broadcast_to([P, N, F])
            nc.vector.tensor_tensor(so3, Xt3, yb, mybir.AluOpType.max)
            for j in range(N):
                nc.tensor.matmul(
                    psum_obs[:],
                    ones[:],
                    so[:, j * F : (j + 1) * F],
                    start=(j == 0),
                    stop=(j == N - 1),
                )

            outt = data.tile([1, 2 * F], f32)
            nc.vector.tensor_copy(outt[:, :F], psum_pairs[:])
            nc.vector.tensor_copy(outt[:, F:], psum_obs[:])
            nc.sync.dma_start(out=out_d[0:1, :], in_=outt[:, :F])
            nc.sync.dma_start(out=out_d[1:2, :], in_=outt[:, F:])

    nc.compile()
    return nc


def _get_nc():
    if "nc" not in _CACHE:
        _CACHE["nc"] = _build()
    return _CACHE["nc"]


def _shard_inputs(forecasts, observations):
    f = np.asarray(forecasts, dtype=np.float32).reshape(N, S_FULL).astype(np.float16)
    o = np.asarray(observations, dtype=np.float32).reshape(S_FULL).astype(np.float16)
    in_maps = []
    for c in range(N_CORES):
        sl = slice(c * S_LOC, (c + 1) * S_LOC)
        in_maps.append(
            {"x": np.ascontiguousarray(f[:, sl]), "y": np.ascontiguousarray(o[sl])}
        )
    return f, o, in_maps


def _combine(f, o, outs):
    """outs: list of per-core [2, F] float32 arrays."""
    U = f.astype(np.float64).sum()
    V = o.astype(np.float64).sum()
    Pm = sum(out[0].astype(np.float64).sum() for out in outs)
    Q = sum(out[1].astype(np.float64).sum() for out in outs)
    T1 = 2.0 * Q - U - N * V
    T2 = 4.0 * Pm + (2.0 - 2.0 * N) * U
    crps = T1 / (N * S_FULL) - T2 / (2.0 * N * N * S_FULL)
    return np.float32(crps)


def kernel(forecasts, observations):
    from concourse.bass_utils import run_bass_kernel_spmd

    nc = _get_nc()
    f, o, in_maps = _shard_inputs(forecasts, observations)
    res = run_bass_kernel_spmd(nc, in_maps, list(range(N_CORES)))
    outs = [res.results[c]["out"] for c in range(N_CORES)]
    return _combine(f, o, outs)


# revision 5
# speedup vs baseline: 1.0879x; 1.0879x over previous
"""CRPS loss kernel for Trainium2 (8 NeuronCores, SPMD).

Math: with |a-b| = 2*max(a,b) - a - b, for forecasts x_i (i<N) and obs y:
  T1 = sum_s sum_i |x_i - y|    = 2*Q - U - N*V
  T2 = sum_s sum_ij |x_i - x_j| = 4*Pm + (2-2N)*U
where
  Pm = sum_s sum_{i<j<N} max(x_i, x_j)   (device)
  Q  = sum_s sum_i max(x_i, y)           (device)
  U  = sum_s sum_i x_i,  V = sum_s y     (host, exact fp64 over fp16 inputs)
and crps_mean = T1/(N*S) - T2/(2*N^2*S).

max() is exact in fp16, so the only precision loss is fp16 input rounding
(measured rel err ~4e-7 vs the fp32 reference).

Device design (per core, spatial shard 65536 pts = [128 part, 512 free]):
- y is treated as member index 20; all 210 unordered pairs (i<j<=20) are
  max-reduced. Pairs with j<20 sum into psum_pairs (-> Pm), pairs with
  j==20 into psum_obs (-> Q).
- Ensemble lives in two SBUF tiles: t0 = members 0-9, t1 = members 10-19
  plus y. Pair (i,j) maxes are batched as contiguous diagonal-segment
  tensor_max ops within/between the tiles, so compute on t0 overlaps the
  DMA of t1.
- Reduction of each 512-col max block runs on the otherwise-idle PE as a
  ones-vector matmul accumulating into PSUM; psum->sbuf copies go on the
  scalar engine; input DMAs are spread over 5 engine queues.
"""

import numpy as np

N_CORES = 8
N = 20
S_FULL = 4 * 1 * 8 * 128 * 128  # 524288
S_LOC = S_FULL // N_CORES  # 65536
P = 128
F = S_LOC // P  # 512
M0 = 10  # members in tile0; tile1 holds members 10..19 plus y (11 blocks)
M1 = N - M0 + 1

_CACHE = {}


def _segments():
    """All pair (i, j=i+d) diagonal segments as (tile_in0, start_block,
    tile_in1, start_block, n_blocks, j_start). Tiles: 0 -> t0, 1 -> t1.
    Virtual member index 20 == y (last block of t1)."""
    segs = []
    NV = N + 1  # 21 virtual members
    for d in range(1, NV):
        # i in t0, j in t0: i <= M0-1-d
        n = M0 - d
        if n > 0:
            segs.append((0, 0, 0, d, n, d))
        # i in t0, j in t1: max(0, M0-d) <= i <= min(M0-1, NV-1-d)
        ilo, ihi = max(0, M0 - d), min(M0 - 1, NV - 1 - d)
        n = ihi - ilo + 1
        if n > 0:
            segs.append((0, ilo, 1, ilo + d - M0, n, ilo + d))
        # i in t1, j in t1: M0 <= i <= NV-1-d
        n = NV - d - M0
        if n > 0:
            segs.append((1, 0, 1, d, n, M0 + d))
    return segs


def _build():
    import concourse.bacc as bacc
    import concourse.tile as tile
    import concourse.mybir as mybir

    f16 = mybir.dt.float16
    f32 = mybir.dt.float32

    nc = bacc.Bacc("TRN2", target_bir_lowering=False, debug=False, num_devices=N_CORES)
    x_d = nc.dram_tensor("x", [N, S_LOC], f16, kind="ExternalInput")
    y_d = nc.dram_tensor("y", [S_LOC], f16, kind="ExternalInput")
    out_d = nc.dram_tensor("out", [2, F], f32, kind="ExternalOutput")

    segs = _segments()
    n_pair_mm = sum(s[4] for s in segs if True) - sum(
        1 for s in segs if s[5] + s[4] - 1 == N
    )  # blocks with j < 20
    # count matmuls per psum target
    total_blocks = sum(s[4] for s in segs)
    n_obs_mm = total_blocks - n_pair_mm

    with tile.TileContext(nc) as tc:
        with (
            tc.tile_pool(name="data", bufs=1) as data,
            tc.tile_pool(name="scr", bufs=4) as scrp,
            tc.tile_pool(name="psum", bufs=1, space="PSUM") as pp,
        ):
            t0 = data.tile([P, M0 * F], f16)
            t1 = data.tile([P, M1 * F], f16)
            ones = data.tile([P, 1], f16)
            nc.vector.memset(ones[:], 1.0)

            x3 = x_d.ap().rearrange("n (p f) -> p n f", p=P)
            t0_3 = t0[:].rearrange("p (n f) -> p n f", f=F)
            t1_3 = t1[:].rearrange("p (n f) -> p n f", f=F)
            # spread input DMAs over the three DMA-capable queues; t0 first
            nc.sync.dma_start(out=t0_3[:, 0:5, :], in_=x3[:, 0:5, :])
            nc.scalar.dma_start(out=t0_3[:, 5:10, :], in_=x3[:, 5:10, :])
            nc.gpsimd.dma_start(out=t1_3[:, 0:5, :], in_=x3[:, 10:15, :])
            nc.sync.dma_start(out=t1_3[:, 5:10, :], in_=x3[:, 15:20, :])
            nc.scalar.dma_start(
                out=t1[:, M0 * F :], in_=y_d.ap().rearrange("(p f) -> p f", p=P)
            )

            psum_pairs = pp.tile([1, F], f32)
            psum_obs = pp.tile([1, F], f32)
            tiles = {0: t0, 1: t1}

            kp = 0
            ko = 0
            for (ta, sa, tb, sb, nblk, j0) in segs:
                L = nblk * F
                s = scrp.tile([P, M0 * F], f16, tag="scr")
                nc.vector.tensor_max(
                    s[:, :L],
                    tiles[ta][:, sa * F : sa * F + L],
                    tiles[tb][:, sb * F : sb * F + L],
                )
                for b in range(nblk):
                    j = j0 + b
                    if j < N:
                        nc.tensor.matmul(
                            psum_pairs[:],
                            ones[:],
                            s[:, b * F : (b + 1) * F],
                            start=(kp == 0),
                            stop=(kp == n_pair_mm - 1),
                            skip_group_check=True,
                        )
                        kp += 1
                    else:
                        nc.tensor.matmul(
                            psum_obs[:],
                            ones[:],
                            s[:, b * F : (b + 1) * F],
                            start=(ko == 0),
                            stop=(ko == n_obs_mm - 1),
                            skip_group_check=True,
                        )
                        ko += 1

            outt = data.tile([1, 2 * F], f32)
            nc.scalar.copy(out=outt[:, :F], in_=psum_pairs[:])
            nc.scalar.copy(out=outt[:, F:], in_=psum_obs[:])
            nc.sync.dma_start(out=out_d[0:1, :], in_=outt[:, :F])
            nc.sync.dma_start(out=out_d[1:2, :], in_=outt[:, F:])

    nc.compile()
    return nc


def _get_nc():
    if "nc" not in _CACHE:
        _CACHE["nc"] = _build()
    return _CACHE["nc"]


def _shard_inputs(forecasts, observations):
    f = np.asarray(forecasts, dtype=np.float32).reshape(N, S_FULL).astype(np.float16)
    o = np.asarray(observations, dtype=np.float32).reshape(S_FULL).astype(np.float16)
    in_maps = []
    for c in range(N_CORES):
        sl = slice(c * S_LOC, (c + 1) * S_LOC)
        in_maps.append(
            {"x": np.ascontiguousarray(f[:, sl]), "y": np.ascontiguousarray(o[sl])}
        )
    return f, o, in_maps


def _combine(f, o, outs):
    """outs: list of per-core [2, F] float32 arrays."""
    U = f.astype(np.float64).sum()
    V = o.astype(np.float64).sum()
    Pm = sum(out[0].astype(np.float64).sum() for out in outs)
    Q = sum(out[1].astype(np.float64).sum() for out in outs)
    T1 = 2.0 * Q - U - N * V
    T2 = 4.0 * Pm + (2.0 - 2.0 * N) * U
    crps = T1 / (N * S_FULL) - T2 / (2.0 * N * N * S_FULL)
    return np.float32(crps)


def kernel(forecasts, observations):
    from concourse.bass_utils import run_bass_kernel_spmd

    nc = _get_nc()
    f, o, in_maps = _shard_inputs(forecasts, observations)
    res = run_bass_kernel_spmd(nc, in_maps, list(range(N_CORES)))
    outs = [res.results[c]["out"] for c in range(N_CORES)]
    return _combine(f, o, outs)


# revision 8
# speedup vs baseline: 1.0934x; 1.0051x over previous
"""CRPS loss kernel for Trainium2 (8 NeuronCores, SPMD).

Math: with |a-b| = 2*max(a,b) - a - b, for forecasts x_i (i<N) and obs y:
  T1 = sum_s sum_i |x_i - y|    = 2*Q - U - N*V
  T2 = sum_s sum_ij |x_i - x_j| = 4*Pm + (2-2N)*U
where
  Pm = sum_s sum_{i<j<N} max(x_i, x_j)   (device)
  Q  = sum_s sum_i max(x_i, y)           (device)
  U  = sum_s sum_i x_i,  V = sum_s y     (host, exact fp64 over fp16 inputs)
and crps_mean = T1/(N*S) - T2/(2*N^2*S).

max() is exact in fp16, so the only precision loss is fp16 input rounding
(measured rel err ~4e-7 vs the fp32 reference).

Device design (per core, spatial shard 65536 pts = [128 part, 512 free]):
- y is treated as member index 20; all 210 unordered pairs (i<j<=20) are
  max-reduced. Pairs with j<20 sum into psum_pairs (-> Pm), pairs with
  j==20 into psum_obs (-> Q).
- Ensemble lives in two SBUF tiles: t0 = members 0-9, t1 = members 10-19
  plus y. Pair (i,j) maxes are batched as contiguous diagonal-segment
  tensor_max ops within/between the tiles, so compute on t0 overlaps the
  DMA of t1.
- Reduction of each 512-col max block runs on the otherwise-idle PE as a
  ones-vector matmul accumulating into PSUM; psum->sbuf copies go on the
  scalar engine; input DMAs are spread over 5 engine queues.
"""

import numpy as np

N_CORES = 8
N = 20
S_FULL = 4 * 1 * 8 * 128 * 128  # 524288
S_LOC = S_FULL // N_CORES  # 65536
P = 128
F = S_LOC // P  # 512
M0 = 10  # members in tile0; tile1 holds members 10..19 plus y (11 blocks)
M1 = N - M0 + 1

_CACHE = {}


def _segments():
    """All pair (i, j=i+d) diagonal segments as (tile_in0, start_block,
    tile_in1, start_block, n_blocks, j_start). Tiles: 0 -> t0, 1 -> t1.
    Virtual member index 20 == y (last block of t1)."""
    segs = []
    NV = N + 1  # 21 virtual members
    for d in range(1, NV):
        # i in t0, j in t0: i <= M0-1-d
        n = M0 - d
        if n > 0:
            segs.append((0, 0, 0, d, n, d))
        # i in t0, j in t1: max(0, M0-d) <= i <= min(M0-1, NV-1-d)
        ilo, ihi = max(0, M0 - d), min(M0 - 1, NV - 1 - d)
        n = ihi - ilo + 1
        if n > 0:
            segs.append((0, ilo, 1, ilo + d - M0, n, ilo + d))
        # i in t1, j in t1: M0 <= i <= NV-1-d
        n = NV - d - M0
        if n > 0:
            segs.append((1, 0, 1, d, n, M0 + d))
    return segs


def _build():
    import concourse.bacc as bacc
    import concourse.tile as tile
    import concourse.mybir as mybir

    f16 = mybir.dt.float16
    f32 = mybir.dt.float32

    nc = bacc.Bacc("TRN2", target_bir_lowering=False, debug=False, num_devices=N_CORES)
    # x is pre-transposed on host to [p, n, f] so DMA rows are contiguous
    x_d = nc.dram_tensor("x", [P, N * F], f16, kind="ExternalInput")
    y_d = nc.dram_tensor("y", [P, F], f16, kind="ExternalInput")
    out_d = nc.dram_tensor("out", [2, F], f32, kind="ExternalOutput")

    segs = _segments()
    n_pair_mm = sum(s[4] for s in segs if True) - sum(
        1 for s in segs if s[5] + s[4] - 1 == N
    )  # blocks with j < 20
    # count matmuls per psum target
    total_blocks = sum(s[4] for s in segs)
    n_obs_mm = total_blocks - n_pair_mm

    with tile.TileContext(nc) as tc:
        with (
            tc.tile_pool(name="data", bufs=1) as data,
            tc.tile_pool(name="scr", bufs=4) as scrp,
            tc.tile_pool(name="psum", bufs=1, space="PSUM") as pp,
        ):
            t0 = data.tile([P, M0 * F], f16)
            t1 = data.tile([P, M1 * F], f16)
            ones = data.tile([P, 1], f16)
            nc.vector.memset(ones[:], 1.0)

            xa = x_d.ap()
            # spread input DMAs over the three DMA-capable queues; t0 first
            nc.sync.dma_start(out=t0[:, : 5 * F], in_=xa[:, : 5 * F])
            nc.scalar.dma_start(out=t0[:, 5 * F :], in_=xa[:, 5 * F : 10 * F])
            nc.gpsimd.dma_start(out=t1[:, : 5 * F], in_=xa[:, 10 * F : 15 * F])
            nc.sync.dma_start(out=t1[:, 5 * F : 10 * F], in_=xa[:, 15 * F :])
            nc.scalar.dma_start(out=t1[:, M0 * F :], in_=y_d.ap())

            psum_pairs = pp.tile([1, F], f32)
            psum_obs = pp.tile([1, F], f32)
            tiles = {0: t0, 1: t1}

            kp = 0
            ko = 0
            for (ta, sa, tb, sb, nblk, j0) in segs:
                L = nblk * F
                s = scrp.tile([P, M0 * F], f16, tag="scr")
                nc.vector.tensor_max(
                    s[:, :L],
                    tiles[ta][:, sa * F : sa * F + L],
                    tiles[tb][:, sb * F : sb * F + L],
                )
                for b in range(nblk):
                    j = j0 + b
                    if j < N:
                        nc.tensor.matmul(
                            psum_pairs[:],
                            ones[:],
                            s[:, b * F : (b + 1) * F],
                            start=(kp == 0),
                            stop=(kp == n_pair_mm - 1),
                            skip_group_check=True,
                        )
                        kp += 1
                    else:
                        nc.tensor.matmul(
                            psum_obs[:],
                            ones[:],
                            s[:, b * F : (b + 1) * F],
                            start=(ko == 0),
                            stop=(ko == n_obs_mm - 1),
                            skip_group_check=True,
                        )
                        ko += 1

            outt = data.tile([1, 2 * F], f32)
            nc.scalar.copy(out=outt[:, :F], in_=psum_pairs[:])
            nc.scalar.copy(out=outt[:, F:], in_=psum_obs[:])
            nc.sync.dma_start(out=out_d[0:1, :], in_=outt[:, :F])
            nc.sync.dma_start(out=out_d[1:2, :], in_=outt[:, F:])

    nc.compile()
    return nc


def _get_nc():
    if "nc" not in _CACHE:
        _CACHE["nc"] = _build()
    return _CACHE["nc"]


def _shard_inputs(forecasts, observations):
    f = np.asarray(forecasts, dtype=np.float32).reshape(N, S_FULL).astype(np.float16)
    o = np.asarray(observations, dtype=np.float32).reshape(S_FULL).astype(np.float16)
    # device layout: [p, n, f] per core so each DMA row is contiguous
    fr = f.reshape(N, N_CORES, P, F)
    orr = o.reshape(N_CORES, P, F)
    in_maps = []
    for c in range(N_CORES):
        xc = np.ascontiguousarray(fr[:, c].transpose(1, 0, 2)).reshape(P, N * F)
        in_maps.append({"x": xc, "y": orr[c]})
    return f, o, in_maps


def _combine(f, o, outs):
    """outs: list of per-core [2, F] float32 arrays."""
    U = f.astype(np.float64).sum()
    V = o.astype(np.float64).sum()
    Pm = sum(out[0].astype(np.float64).sum() for out in outs)
    Q = sum(out[1].astype(np.float64).sum() for out in outs)
    T1 = 2.0 * Q - U - N * V
    T2 = 4.0 * Pm + (2.0 - 2.0 * N) * U
    crps = T1 / (N * S_FULL) - T2 / (2.0 * N * N * S_FULL)
    return np.float32(crps)


def kernel(forecasts, observations):
    from concourse.bass_utils import run_bass_kernel_spmd

    nc = _get_nc()
    f, o, in_maps = _shard_inputs(forecasts, observations)
    res = run_bass_kernel_spmd(nc, in_maps, list(range(N_CORES)))
    outs = [res.results[c]["out"] for c in range(N_CORES)]
    return _combine(f, o, outs)
